# revision 30
# baseline (speedup 1.0000x reference)
"""Trainium2 Bass kernel for nn_FLD_83236466197026 (dense_transformer).

Strategy: data-parallel over batch B=64 across 8 cores (8 batches/core).

Algebraic restructuring (validated on host against the fp32 reference):
  * scores = sin(t*ws+bs) @ As + t*c1, with As/c1 folded from
    W_k/query/W_q on host (softmax-ratio invariance drops the constant
    term and the max-subtraction; |scores| < 4 on this data).
  * The t*c1 affine term is ONE K=16 matmul: block-diag c1big against
    t reshaped [16, 128], accumulated into the scores PSUM, so exp
    reads complete pre-exp scores straight from PSUM.
  * V = [M*X, M] is precomputed host-side in fp8e4 and laid out
    partition-major so each batch's V is one contiguous DMA; num and
    den come from ONE accumulated DoubleRow fp8 matmul chain.
  * x[..., D:] == 1 exactly (mask halves equal), so only W_o's X-half
    is used; W_o @ W1 is folded on host (skips the LAT intermediate).
  * z = c0 + t*c1 + t^2*c2 folds into the first MLP layer evaluated
    transposed: h1 = relu(C1_b.T @ [1; t; t^2] + b1); [1;t;t^2] rows
    for all batches are host-built (Tm).
  * Fully software-pipelined steps: step s runs attention+C1 of batch
    s+1 and h1 of batch s interleaved into the h2/out matmul stream of
    batch s-1, so the PE stays dense and hot the whole kernel.
  * Output is produced transposed [D, T] in fp16; host unshards.

Matmul operands fp16 except num/den (fp8 DoubleRow); PSUM fp32.
Host-simulated end-to-end rel err ~1.3e-3 (gate 2e-2).
"""

import sys

if "/opt/trn_rl_repo" not in sys.path:
    sys.path.insert(0, "/opt/trn_rl_repo")

import numpy as np

N_CORES = 8
B, L, T, D = 64, 2048, 1024, 128
E, H, P = 512, 8, 3
LAT, HID = 256, 512
NB = B // N_CORES       # batches per core
NS = E // H             # sin channels (64)
J = H * P               # flattened (head, poly) dim (24)
NCH = L // 128          # l-chunks per batch (16)
NG = NCH // 2           # chunk pairs (8)
HL = L // 2             # half length (1024)

_PROG_CACHE = {}


def _build_program(nb=NB, phase=3):
    """Build (once) the single-core Bass/Tile program shared by all cores."""
    import concourse.bacc as bacc
    import concourse.mybir as mybir
    from concourse.tile import TileContext, add_dep_helper

    dt = mybir.dt
    AF = mybir.ActivationFunctionType
    ALU = mybir.AluOpType
    DRm = mybir.MatmulPerfMode.DoubleRow
    f32, f16, f8 = dt.float32, dt.float16, dt.float8e4

    nc = bacc.Bacc("TRN2", target_bir_lowering=False, debug=False,
                   num_devices=N_CORES)

    # ---- DRAM I/O ----
    t2r_d = nc.dram_tensor("t2r", [nb, 2, HL], f16, kind="ExternalInput")
    V_d = nc.dram_tensor("V", [nb, 128, NG * 2 * 2 * D], f8,
                         kind="ExternalInput")
    Tm_d = nc.dram_tensor("Tm", [P, nb * T], f16, kind="ExternalInput")
    As_d = nc.dram_tensor("As", [128, 2 * J], f16, kind="ExternalInput")
    wsbs_d = nc.dram_tensor("wsbs", [128, 2], f32, kind="ExternalInput")
    c1b_d = nc.dram_tensor("c1b", [NCH, NCH * J], f16, kind="ExternalInput")
    Wox1_d = nc.dram_tensor("Wox1", [128, H * HID], f16, kind="ExternalInput")
    beff1_d = nc.dram_tensor("beff1", [1, HID], f16, kind="ExternalInput")
    W2_d = nc.dram_tensor("W2", [128, 4 * HID], f16, kind="ExternalInput")
    W3_d = nc.dram_tensor("W3", [128, 4 * D], f16, kind="ExternalInput")
    b1_d = nc.dram_tensor("b1", [128, HID // 128], f32, kind="ExternalInput")
    b2_d = nc.dram_tensor("b2", [128, HID // 128], f32, kind="ExternalInput")
    b3_d = nc.dram_tensor("b3", [128, 1], f32, kind="ExternalInput")
    eye_d = nc.dram_tensor("eye", [J, J], f16, kind="ExternalInput")
    o_d = nc.dram_tensor("o", [nb, D, T], f16, kind="ExternalOutput")

    with TileContext(nc) as tc:
        with (
            tc.tile_pool(name="pconst", bufs=1) as pc,
            tc.tile_pool(name="ptb", bufs=4) as ptb,
            tc.tile_pool(name="psin", bufs=nb // 2) as psin,
            tc.tile_pool(name="pt16", bufs=3) as pt16,
            tc.tile_pool(name="pv", bufs=3) as pv,
            tc.tile_pool(name="pw", bufs=2) as pw,
            tc.tile_pool(name="psm", bufs=2) as psm,
            tc.tile_pool(name="pc1", bufs=3) as pc1,
            tc.tile_pool(name="ph1", bufs=2) as ph1,
            tc.tile_pool(name="ph2", bufs=2) as ph2,
            tc.tile_pool(name="pout", bufs=2) as pout,
            tc.tile_pool(name="ps", bufs=1, space="PSUM") as pp,
        ):
            # ---- constants (sin prerequisites first, heavy weights on
            # the gpsimd queue after the time-critical tb broadcasts) ----
            wsbs_sb = pc.tile([128, 2], f32, tag="wsbs")
            nc.sync.dma_start(out=wsbs_sb[:], in_=wsbs_d[:])

            # sin pair tiles: cols [0:HL] = batch 2p, [HL:2HL] = batch 2p+1.
            # tb broadcasts go on sync (pairs 0-1) and vector (pairs 2-3)
            # so the gpsimd queue is free for V8/t16 prefetch + weights:
            # every sin gates exp(0) via the table-set dep, so the whole
            # set is on the critical-path prefix.
            t16s, V8s = {}, {}
            for _b in (0, 1):
                _tt = pt16.tile([NCH, 128], f16, tag="t16",
                                name=f"t16_{_b}")
                nc.gpsimd.dma_start(
                    out=_tt[:],
                    in_=t2r_d[_b].rearrange("r (g l) -> (r g) l", l=128))
                t16s[_b] = _tt

            tbs, sins = [], []
            tbt = []
            for p in range(nb // 2):
                tb = ptb.tile([128, 2 * HL], f16, tag="tb")
                b0, b1 = 2 * p, 2 * p + 1
                if p < 2:
                    # latency-critical pairs: 32-way half broadcasts
                    hn = NS // 2
                    for q, (bb, r, pbase) in enumerate(
                            [(b0, 0, 0), (b0, 1, NS), (b1, 0, 0),
                             (b1, 1, NS)]):
                        col = slice(0, HL) if bb == b0 else slice(HL, 2 * HL)
                        for hh in range(2):
                            eng = nc.sync if (q + hh) % 2 == 0 else nc.gpsimd
                            eng.dma_start(
                                out=tb[pbase + hn * hh:pbase + hn * (hh + 1),
                                       col],
                                in_=t2r_d[bb, r].partition_broadcast(hn))
                else:
                    nc.sync.dma_start(out=tb[0:NS, 0:HL],
                                      in_=t2r_d[b0, 0].partition_broadcast(NS))
                    nc.sync.dma_start(out=tb[NS:128, 0:HL],
                                      in_=t2r_d[b0, 1].partition_broadcast(NS))
                    nc.gpsimd.dma_start(
                        out=tb[0:NS, HL:2 * HL],
                        in_=t2r_d[b1, 0].partition_broadcast(NS))
                    nc.gpsimd.dma_start(
                        out=tb[NS:128, HL:2 * HL],
                        in_=t2r_d[b1, 1].partition_broadcast(NS))
                tbt.append(tb)
            def emit_sin(p):
                st = psin.tile([128, 2 * HL], f16, tag="sinT",
                               name=f"sinT_{p}")
                sins.append(nc.scalar.activation(st[:], tbt[p][:], AF.Sin,
                                                 bias=wsbs_sb[:, 1:2],
                                                 scale=wsbs_sb[:, 0:1]))
                tbs.append(st)

            for p in range(nb // 2):
                emit_sin(p)
            As_sb = pc.tile([128, 2 * J], f16, tag="As")
            nc.sync.dma_start(out=As_sb[:], in_=As_d[:])
            c1b_sb = pc.tile([NCH, NCH * J], f16, tag="c1b")
            nc.sync.dma_start(out=c1b_sb[:], in_=c1b_d[:])
            eye_sb = pc.tile([J, J], f16, tag="eye")
            nc.sync.dma_start(out=eye_sb[:], in_=eye_d[:])

            # ---- prefetches (distance 2) ----

            def prefetch(b, eng=None):
                if b >= nb:
                    return
                if b not in t16s:
                    tt = pt16.tile([NCH, 128], f16, tag="t16",
                                   name=f"t16_{b}")
                    nc.gpsimd.dma_start(
                        out=tt[:],
                        in_=t2r_d[b].rearrange("r (g l) -> (r g) l", l=128))
                    t16s[b] = tt
                V8 = pv.tile([128, NG * 2 * 2 * D], f8, tag="V8")
                if eng is None:
                    eng = nc.sync if b % 2 == 0 else nc.gpsimd
                eng.dma_start(out=V8[:], in_=V_d[b])
                V8s[b] = V8

            prefetch(0, eng=nc.gpsimd)
            prefetch(1, eng=nc.gpsimd)

            # heavy constants (needed from the first C1/h1 onward)
            Wox1_sb = pc.tile([128, H * HID], f16, tag="Wox1")
            nc.gpsimd.dma_start(out=Wox1_sb[:], in_=Wox1_d[:])
            beff1_sb = pc.tile([1, HID], f16, tag="beff1")
            nc.gpsimd.dma_start(out=beff1_sb[:], in_=beff1_d[:])
            Tm_sb = pc.tile([P, nb * T], f16, tag="Tm")
            nc.sync.dma_start(out=Tm_sb[:], in_=Tm_d[:])
            W2_sb = pc.tile([128, 4 * HID], f16, tag="W2")
            nc.gpsimd.dma_start(out=W2_sb[:], in_=W2_d[:])
            W3_sb = pc.tile([128, 4 * D], f16, tag="W3")
            nc.gpsimd.dma_start(out=W3_sb[:], in_=W3_d[:])
            b1_sb = pc.tile([128, HID // 128], f32, tag="b1")
            nc.gpsimd.dma_start(out=b1_sb[:], in_=b1_d[:])
            b2_sb = pc.tile([128, HID // 128], f32, tag="b2")
            nc.gpsimd.dma_start(out=b2_sb[:], in_=b2_d[:])
            b3_sb = pc.tile([128, 1], f32, tag="b3")
            nc.gpsimd.dma_start(out=b3_sb[:], in_=b3_d[:])
            ones24 = pc.tile([1, J], f16, tag="ones24")
            nc.vector.memset(ones24[:], 1.0)

            C1s = {}
            xTp = [None]

            def att_block(b):
                """scores -> exp -> num/den -> x -> xT -> C1_b for batch b.
                PE parts are split so exp/DVE latency hides under the h2
                stream the caller interleaves around them."""
                st = tbs[b // 2]
                off = HL * (b % 2)
                # scores: c1big opens the accumulation (t*c1 term, one
                # K=16 matmul), then 8 sin-part matmuls close per block.
                ps_s = pp.tile([128, NCH * J], f32, tag="ps_s", bufs=2,
                               name=f"ps_s_{b}")
                nc.tensor.matmul(ps_s[:], t16s[b][:], c1b_sb[:],
                                 start=True, stop=False,
                                 skip_group_check=True)
                for g in range(NG):
                    nc.tensor.matmul(ps_s[:, 2 * J * g:2 * J * (g + 1)],
                                     st[:, off + 128 * g:off + 128 * (g + 1)],
                                     As_sb[:], start=False, stop=True,
                                     skip_group_check=True)
                # w8 pads each 24-col chunk block to 32 so the DoubleRow
                # ldweights k-pair step is 16B-aligned (s3_lw restriction).
                w8 = pw.tile([128, NG * 2 * 32], f8, tag="w8")
                w8v = w8[:].rearrange("p (g k j) -> p g k j",
                                      g=NG, k=2)[:, :, :, 0:J]
                exp_i = nc.scalar.activation(
                    w8v, ps_s[:].rearrange("p (g k j) -> p g k j", g=NG, k=2),
                    AF.Exp)
                add_dep_helper(exp_i.ins, sins[-1].ins, sync=False,
                               reason="sin table set before exp set")

                def nd_block(b=b, w8v=w8v):
                    ps_nd = pp.tile([J, 2 * D], f32, tag="ps_nd", bufs=1,
                                    name=f"ps_nd_{b}")
                    V8v = V8s.pop(b)[:].rearrange("p (g k c) -> p g k c",
                                                  g=NG, k=2)
                    for g in range(NG):
                        nc.tensor.matmul(ps_nd[:], w8v[:, g], V8v[:, g],
                                         start=(g == 0), stop=(g == NG - 1),
                                         perf_mode=DRm)
                    rden = psm.tile([J, D], f32, tag="rden")
                    nc.vector.reciprocal(rden[:], ps_nd[:, D:2 * D])
                    x16 = psm.tile([J, D], f16, tag="x16")
                    nc.vector.tensor_mul(x16[:], ps_nd[:, 0:D], rden[:])
                    return x16

                def xt_block(x16, b=b):
                    ps_xt = pp.tile([D, J], f16, tag="ps_c1", bufs=1,
                                    name=f"ps_xt_{b}")
                    nc.tensor.transpose(ps_xt[:], x16[:], eye_sb[:])
                    if b % 2 == 0:
                        xTp[0] = psm.tile([D, 2 * J], f16, tag="xTp", name=f"xTp_{b}")
                    dst = xTp[0][:].rearrange("p (h c q) -> p h c q",
                                              h=H, c=2)[:, :, b % 2, :]
                    nc.vector.tensor_copy(
                        dst, ps_xt[:].rearrange("p (h q) -> p h q", h=H))

                def c1_block(b=b):
                    # C1 for the pair (b-1, b): [6, HID], rows (batch, p)
                    ps_c1 = pp.tile([2 * P, HID], f32, tag="ps_c1", bufs=1,
                                    name=f"ps_c1_{b}")
                    for h in range(H):
                        nc.tensor.matmul(ps_c1[:],
                                         xTp[0][:, 2 * P * h:2 * P * (h + 1)],
                                         Wox1_sb[:, HID * h:HID * (h + 1)],
                                         start=(h == 0), stop=False)
                    nc.tensor.matmul(ps_c1[:], ones24[:, 0:2 * P],
                                     beff1_sb[:], start=False, stop=True)
                    cp = psm.tile([2 * P, HID], f16, tag="C1p", name=f"C1p_{b}")
                    nc.vector.tensor_copy(cp[:], ps_c1[:])
                    for i, bb in enumerate((b - 1, b)):
                        cb = pc1.tile([P, HID], f16, tag=f"C1_{bb % 3}", name=f"C1_{bb}")
                        eng = nc.sync if i == 0 else nc.gpsimd
                        eng.dma_start(out=cb[:],
                                      in_=cp[P * i:P * (i + 1), :])
                        C1s[bb] = cb

                return nd_block, xt_block, (c1_block if b % 2 == 1 else None)

            # ---- fully pipelined steps ----
            h1_cur = None
            h1_prev = None
            # prologue (replaces steps -1 and 0): prefire scores/exp of
            # batches 0 AND 1 before any nd work so the exp chain and the
            # ps_nd-serialized recip chains overlap instead of stacking.
            prefetch(2)
            nd0, xt0, _ = att_block(0)
            nd1, xt1, c1p0 = att_block(1)
            prefetch(3)
            xt0(nd0())
            xt1(nd1())
            c1p0()
            nd_fn = xt_fn = c1_fn = None
            for s in range(1, nb + 2):
                ba, bh1, bh2 = s + 1, s - 1, s - 2
                prefetch(s + 2)
                # attention part 1 of batch s+1 (scores + exp issued)
                if ba <= nb - 1:
                    nd_fn, xt_fn, c1_fn = att_block(ba)
                else:
                    nd_fn = xt_fn = c1_fn = None
                if 0 <= bh1 < nb:
                    h1_cur = [ph1.tile([128, T], f16, tag=f"h1_{m}", bufs=2,
                                       name=f"h1_{bh1}_{m}")
                              for m in range(4)]

                def h1_job(i, bh1=bh1, h1_cur=h1_cur):
                    m, tg = i // 2, i % 2
                    ps_h1 = pp.tile([128, 512], f32, tag="ps_big1", bufs=2,
                                    name=f"ps_h1_{bh1}_{i}")
                    nc.tensor.matmul(
                        ps_h1[:], C1s[bh1][:, 128 * m:128 * (m + 1)],
                        Tm_sb[:, T * bh1 + 512 * tg:T * bh1 + 512 * (tg + 1)],
                        start=True, stop=True)
                    dstv = h1_cur[m][:, 512 * tg:512 * (tg + 1)]
                    if i % 2 == 0:
                        nc.vector.tensor_scalar(dstv, ps_h1[:],
                                                b1_sb[:, m:m + 1], 0.0,
                                                ALU.add, ALU.max)
                    else:
                        nc.scalar.activation(dstv, ps_h1[:], AF.Relu,
                                             bias=b1_sb[:, m:m + 1])

                def h2_group(m, tg, bh2=bh2, h1_prev=h1_prev):
                    ps_h2 = pp.tile([128, 512], f32, tag="ps_big2",
                                    bufs=2, name=f"ps_h2_{bh2}_{m}_{tg}")
                    for k in range(4):
                        nc.tensor.matmul(
                            ps_h2[:],
                            W2_sb[:, HID * k + 128 * m:
                                  HID * k + 128 * (m + 1)],
                            h1_prev[k][:, 512 * tg:512 * (tg + 1)],
                            start=(k == 0), stop=(k == 3))
                    nc.scalar.activation(
                        h2s[m][:, 512 * tg:512 * (tg + 1)], ps_h2[:],
                        AF.Relu, bias=b2_sb[:, m:m + 1])

                if bh2 < 0:
                    # pipeline fill: no h2 stream yet
                    if nd_fn is not None:
                        xt_fn(nd_fn())
                        if c1_fn is not None:
                            c1_fn()
                    if 0 <= bh1 < nb:
                        for i in range(8):
                            h1_job(i)
                    h1_prev = h1_cur
                    continue

                h2s = [ph2.tile([128, T], f16, tag=f"h2_{m}", bufs=2,
                                name=f"h2_{bh2}_{m}") for m in range(4)]
                o_sb = pout.tile([128, T], f16, tag="o_sb", name=f"o3_{bh2}")

                def out_group(tg, bh2=bh2, o_sb=o_sb):
                    ps_o = pp.tile([128, 512], f32, tag="ps_big1",
                                   bufs=2, name=f"ps_o_{bh2}_{tg}")
                    for k in range(4):
                        nc.tensor.matmul(
                            ps_o[:], W3_sb[:, D * k:D * (k + 1)],
                            h2s[k][:, 512 * tg:512 * (tg + 1)],
                            start=(k == 0), stop=(k == 3))
                    nc.vector.tensor_scalar_add(
                        o_sb[:, 512 * tg:512 * (tg + 1)], ps_o[:],
                        b3_sb[:, 0:1])
                    nc.sync.dma_start(out=o_d[bh2, :, 512 * tg:512 * (tg + 1)],
                                      in_=o_sb[:, 512 * tg:512 * (tg + 1)])

                if s == nb + 1:
                    # drain step: tg-major so the out matmuls of tg=0
                    # overlap the h2 matmuls of tg=1
                    for tg in range(2):
                        for m in range(4):
                            h2_group(m, tg)
                        out_group(tg)
                    h1_prev = h1_cur
                    continue
                # first half of the h2 stream, h1 jobs interleaved in
                # stationary-sharing pairs (halves K-geometry switches)
                for m in range(2):
                    for tg in range(2):
                        h2_group(m, tg)
                    if 0 <= bh1 < nb:
                        h1_job(2 * m)
                        h1_job(2 * m + 1)
                # attention part 2 (nd needs exp, which ran during the
                # h2 groups above); the xt transpose + C1 matmuls go one
                # h2 group later so the recip/x16 DVE latency hides
                x16v = nd_fn() if nd_fn is not None else None
                for m in range(2, 4):
                    for tg in range(2):
                        h2_group(m, tg)
                    if m == 2 and x16v is not None:
                        xt_fn(x16v)
                        if c1_fn is not None:
                            c1_fn()
                    if 0 <= bh1 < nb:
                        h1_job(2 * m)
                        h1_job(2 * m + 1)
                # out^T [D, T] = W3.T @ h2 + b3 (DVE eviction, fp16)
                for tg in range(2):
                    out_group(tg)
                h1_prev = h1_cur

    nc.compile()
    return nc


def _fold_params(inp):
    """Host-side parameter folding (float64 for exactness, cast at the end)."""
    f8d = np.float64
    q = inp["query"][0].astype(f8d) @ inp["W_q"].astype(f8d) + inp["b_q"].astype(f8d)
    Wk = inp["W_k"].astype(f8d)
    ek = E // H
    A = np.zeros((E, J))
    for h in range(H):
        cols = slice(h * ek, (h + 1) * ek)
        for p in range(P):
            A[:, h * P + p] = Wk[:, cols] @ q[p, cols]
    A /= np.sqrt(ek)
    sinm = (np.arange(E) % H) == 0
    ws = inp["w_te"].astype(f8d)[sinm]
    bs = inp["b_te"].astype(f8d)[sinm]
    As = A[sinm]
    c1 = inp["w_te"].astype(f8d)[~sinm] @ A[~sinm]
    # NOTE: the per-j constant (b_te part + b_k part) cancels in num/den.
    Wo = inp["W_o"].astype(f8d)
    Wox = np.zeros((H * D, LAT))
    beff = inp["b_o"].astype(f8d).copy()
    for h in range(H):
        Wox[h * D:(h + 1) * D] = Wo[h * 2 * D:h * 2 * D + D]
        beff += Wo[h * 2 * D + D:(h + 1) * 2 * D].sum(axis=0)
    W1 = inp["W1"].astype(f8d)
    Wox1 = Wox @ W1                                   # [H*D, HID]
    beff1 = beff @ W1                                 # [HID]
    As2 = np.zeros((128, 2 * J))
    As2[0:NS, 0:J] = As
    As2[NS:128, J:2 * J] = As
    # c1big: row i = chunk i (l in [128i, 128(i+1))), block-diag c1 at
    # the ps_s column block of chunk i: 48*(i%8) + 24*(i//8).
    c1big = np.zeros((NCH, NCH * J))
    for i in range(NCH):
        base = 2 * J * (i % NG) + J * (i // NG)
        c1big[i, base:base + J] = c1
    Wox1_sb = np.zeros((128, H * HID))
    for h in range(H):
        Wox1_sb[:, HID * h:HID * (h + 1)] = Wox1[128 * h:128 * (h + 1), :]
    W2_sb = np.zeros((128, 4 * HID))
    for k in range(4):
        W2_sb[:, HID * k:HID * (k + 1)] = inp["W2"][128 * k:128 * (k + 1), :]
    W3_sb = np.zeros((128, 4 * D))
    for k in range(4):
        W3_sb[:, D * k:D * (k + 1)] = inp["W3"][128 * k:128 * (k + 1), :]
    return {
        "As": As2.astype(np.float16),
        "wsbs": np.stack([np.concatenate([ws, ws]),
                          np.concatenate([bs, bs])], axis=1).astype(np.float32),
        "c1b": c1big.astype(np.float16),
        "Wox1": Wox1_sb.astype(np.float16),
        "beff1": beff1.astype(np.float16)[None, :],
        "W2": W2_sb.astype(np.float16),
        "W3": W3_sb.astype(np.float16),
        "b1": np.ascontiguousarray(
            inp["b1"].astype(np.float32).reshape(HID // 128, 128).T),
        "b2": np.ascontiguousarray(
            inp["b2"].astype(np.float32).reshape(HID // 128, 128).T),
        "b3": inp["b3"].astype(np.float32)[:, None],
        "eye": np.eye(J, dtype=np.float16),
    }


def kernel(**inputs):
    import ml_dtypes
    from concourse.bass_utils import run_bass_kernel_spmd

    if "prog" not in _PROG_CACHE:
        _PROG_CACHE["prog"] = _build_program(
            phase=_PROG_CACHE.get("phase", 3))
    nc = _PROG_CACHE["prog"]

    inp = {k: np.asarray(v) for k, v in inputs.items()}
    params = _fold_params(inp)

    t16 = inp["timesteps"].astype(np.float16)            # [B, L]
    y16 = inp["y_time_steps"].astype(np.float16)         # [B, T]
    t2y = (y16.astype(np.float32) ** 2).astype(np.float16)
    # V = [M*X, M] packed [b, p, g, half, c] so l = 128*(g + 8*half) + p
    Vf = np.concatenate([inp["M"] * inp["X"], inp["M"]], axis=-1)  # [B,L,2D]
    Vp = Vf.reshape(B, 2, NG, 128, 2 * D).transpose(0, 3, 2, 1, 4)
    V8 = np.ascontiguousarray(Vp.reshape(B, 128, NG * 2 * 2 * D)).astype(
        ml_dtypes.float8_e4m3)

    in_maps = []
    for c in range(N_CORES):
        sl = slice(NB * c, NB * (c + 1))
        ones = np.ones((1, NB * T), np.float16)
        m = {
            "t2r": np.ascontiguousarray(t16[sl].reshape(NB, 2, L // 2)),
            "V": V8[sl],
            "Tm": np.concatenate(
                [ones, y16[sl].reshape(1, -1), t2y[sl].reshape(1, -1)],
                axis=0),
        }
        m.update(params)
        in_maps.append(m)

    res = run_bass_kernel_spmd(nc, in_maps, list(range(N_CORES)),
                               **_PROG_CACHE.get("run_kwargs", {}))
    _PROG_CACHE["last_results"] = res
    out = np.empty((B, T, D), np.float32)
    for c in range(N_CORES):
        out[NB * c:NB * (c + 1)] = (
            res.results[c]["o"].astype(np.float32).transpose(0, 2, 1))
    return out


# revision 31
# speedup vs baseline: 1.2416x; 1.2416x over previous
"""Trainium2 Bass kernel for nn_FLD_83236466197026 (dense_transformer).

Strategy: data-parallel over batch B=64 across 8 cores (8 batches/core).

Algebraic restructuring (validated on host against the fp32 reference):
  * scores = sin(t*ws+bs) @ As + t*c1, with As/c1 folded from
    W_k/query/W_q on host (softmax-ratio invariance drops the constant
    term and the max-subtraction; |scores| < 4 on this data).
  * The t*c1 affine term is ONE K=16 matmul: block-diag c1big against
    t reshaped [16, 128], accumulated into the scores PSUM, so exp
    reads complete pre-exp scores straight from PSUM.
  * V = [M*X, M] is precomputed host-side in fp8e4 and laid out
    partition-major so each batch's V is one contiguous DMA; num and
    den come from ONE accumulated DoubleRow fp8 matmul chain.
  * x[..., D:] == 1 exactly (mask halves equal), so only W_o's X-half
    is used; W_o @ W1 is folded on host (skips the LAT intermediate).
  * z = c0 + t*c1 + t^2*c2 folds into the first MLP layer evaluated
    transposed: h1 = relu(C1_b.T @ [1; t; t^2] + b1); [1;t;t^2] rows
    for all batches are host-built (Tm).
  * Fully software-pipelined steps: step s runs attention+C1 of batch
    s+1 and h1 of batch s interleaved into the h2/out matmul stream of
    batch s-1, so the PE stays dense and hot the whole kernel.
  * Output is produced transposed [D, T] in fp16; host unshards.

Matmul operands fp16 except num/den (fp8 DoubleRow); PSUM fp32.
Host-simulated end-to-end rel err ~1.3e-3 (gate 2e-2).
"""

import sys

if "/opt/trn_rl_repo" not in sys.path:
    sys.path.insert(0, "/opt/trn_rl_repo")

import numpy as np

N_CORES = 8
B, L, T, D = 64, 2048, 1024, 128
E, H, P = 512, 8, 3
LAT, HID = 256, 512
NB = B // N_CORES       # batches per core
NS = E // H             # sin channels (64)
J = H * P               # flattened (head, poly) dim (24)
NCH = L // 128          # l-chunks per batch (16)
NG = NCH // 2           # chunk pairs (8)
HL = L // 2             # half length (1024)

_PROG_CACHE = {}


def _build_program(nb=NB, phase=3):
    """Build (once) the single-core Bass/Tile program shared by all cores."""
    import concourse.bacc as bacc
    import concourse.mybir as mybir
    from concourse.tile import TileContext, add_dep_helper

    dt = mybir.dt
    AF = mybir.ActivationFunctionType
    ALU = mybir.AluOpType
    DRm = mybir.MatmulPerfMode.DoubleRow
    f32, f16, f8 = dt.float32, dt.float16, dt.float8e4

    nc = bacc.Bacc("TRN2", target_bir_lowering=False, debug=False,
                   num_devices=N_CORES)

    # ---- DRAM I/O ----
    t2r_d = nc.dram_tensor("t2r", [nb, 2, HL], f16, kind="ExternalInput")
    V_d = nc.dram_tensor("V", [nb, 128, NG * 2 * 2 * D], f8,
                         kind="ExternalInput")
    Tm_d = nc.dram_tensor("Tm", [P, nb * T], f16, kind="ExternalInput")
    As_d = nc.dram_tensor("As", [128, 2 * J], f16, kind="ExternalInput")
    wsbs_d = nc.dram_tensor("wsbs", [128, 2], f32, kind="ExternalInput")
    c1b_d = nc.dram_tensor("c1b", [NCH, NCH * J], f16, kind="ExternalInput")
    Wox1_d = nc.dram_tensor("Wox1", [128, H * HID], f16, kind="ExternalInput")
    beff1_d = nc.dram_tensor("beff1", [1, HID], f16, kind="ExternalInput")
    W2_d = nc.dram_tensor("W2", [128, 4 * HID], f16, kind="ExternalInput")
    W3_d = nc.dram_tensor("W3", [128, 4 * D], f16, kind="ExternalInput")
    b1_d = nc.dram_tensor("b1", [128, HID // 128], f32, kind="ExternalInput")
    b2_d = nc.dram_tensor("b2", [128, HID // 128], f32, kind="ExternalInput")
    b3_d = nc.dram_tensor("b3", [128, 1], f32, kind="ExternalInput")
    eye_d = nc.dram_tensor("eye", [J, J], f16, kind="ExternalInput")
    o_d = nc.dram_tensor("o", [nb, D, T], f16, kind="ExternalOutput")

    with TileContext(nc) as tc:
        with (
            tc.tile_pool(name="pconst", bufs=1) as pc,
            tc.tile_pool(name="ptb", bufs=4) as ptb,
            tc.tile_pool(name="psin", bufs=nb // 2) as psin,
            tc.tile_pool(name="pt16", bufs=3) as pt16,
            tc.tile_pool(name="pv", bufs=3) as pv,
            tc.tile_pool(name="pw", bufs=2) as pw,
            tc.tile_pool(name="psm", bufs=2) as psm,
            tc.tile_pool(name="pc1", bufs=3) as pc1,
            tc.tile_pool(name="ph1", bufs=2) as ph1,
            tc.tile_pool(name="ph2", bufs=2) as ph2,
            tc.tile_pool(name="pout", bufs=2) as pout,
            tc.tile_pool(name="ps", bufs=1, space="PSUM") as pp,
        ):
            # ---- constants (sin prerequisites first, heavy weights on
            # the gpsimd queue after the time-critical tb broadcasts) ----
            wsbs_sb = pc.tile([128, 2], f32, tag="wsbs")
            nc.sync.dma_start(out=wsbs_sb[:], in_=wsbs_d[:])

            # sin pair tiles: cols [0:HL] = batch 2p, [HL:2HL] = batch 2p+1.
            # tb broadcasts go on sync (pairs 0-1) and vector (pairs 2-3)
            # so the gpsimd queue is free for V8/t16 prefetch + weights:
            # every sin gates exp(0) via the table-set dep, so the whole
            # set is on the critical-path prefix.
            t16s, V8s = {}, {}
            for _b in (0, 1):
                _tt = pt16.tile([NCH, 128], f16, tag="t16",
                                name=f"t16_{_b}")
                nc.gpsimd.dma_start(
                    out=_tt[:],
                    in_=t2r_d[_b].rearrange("r (g l) -> (r g) l", l=128))
                t16s[_b] = _tt

            tbs, sins = [], []
            tbt = []
            for p in range(nb // 2):
                tb = ptb.tile([128, 2 * HL], f16, tag="tb")
                b0, b1 = 2 * p, 2 * p + 1
                if p < 2:
                    # latency-critical pairs: 32-way half broadcasts
                    hn = NS // 2
                    for q, (bb, r, pbase) in enumerate(
                            [(b0, 0, 0), (b0, 1, NS), (b1, 0, 0),
                             (b1, 1, NS)]):
                        col = slice(0, HL) if bb == b0 else slice(HL, 2 * HL)
                        for hh in range(2):
                            eng = nc.sync if (q + hh) % 2 == 0 else nc.gpsimd
                            eng.dma_start(
                                out=tb[pbase + hn * hh:pbase + hn * (hh + 1),
                                       col],
                                in_=t2r_d[bb, r].partition_broadcast(hn))
                else:
                    nc.sync.dma_start(out=tb[0:NS, 0:HL],
                                      in_=t2r_d[b0, 0].partition_broadcast(NS))
                    nc.sync.dma_start(out=tb[NS:128, 0:HL],
                                      in_=t2r_d[b0, 1].partition_broadcast(NS))
                    nc.gpsimd.dma_start(
                        out=tb[0:NS, HL:2 * HL],
                        in_=t2r_d[b1, 0].partition_broadcast(NS))
                    nc.gpsimd.dma_start(
                        out=tb[NS:128, HL:2 * HL],
                        in_=t2r_d[b1, 1].partition_broadcast(NS))
                tbt.append(tb)
            def emit_sin(p):
                st = psin.tile([128, 2 * HL], f16, tag="sinT",
                               name=f"sinT_{p}")
                sins.append(nc.scalar.activation(st[:], tbt[p][:], AF.Sin,
                                                 bias=wsbs_sb[:, 1:2],
                                                 scale=wsbs_sb[:, 0:1]))
                tbs.append(st)

            for p in range(nb // 2):
                emit_sin(p)
            As_sb = pc.tile([128, 2 * J], f16, tag="As")
            nc.sync.dma_start(out=As_sb[:], in_=As_d[:])
            c1b_sb = pc.tile([NCH, NCH * J], f16, tag="c1b")
            nc.sync.dma_start(out=c1b_sb[:], in_=c1b_d[:])
            eye_sb = pc.tile([J, J], f16, tag="eye")
            nc.sync.dma_start(out=eye_sb[:], in_=eye_d[:])

            # ---- prefetches (distance 2) ----

            def prefetch(b, eng=None):
                if b >= nb:
                    return
                if b not in t16s:
                    tt = pt16.tile([NCH, 128], f16, tag="t16",
                                   name=f"t16_{b}")
                    nc.gpsimd.dma_start(
                        out=tt[:],
                        in_=t2r_d[b].rearrange("r (g l) -> (r g) l", l=128))
                    t16s[b] = tt
                V8 = pv.tile([128, NG * 2 * 2 * D], f8, tag="V8")
                if eng is None:
                    eng = nc.sync if b % 2 == 0 else nc.gpsimd
                eng.dma_start(out=V8[:], in_=V_d[b])
                V8s[b] = V8

            prefetch(0, eng=nc.gpsimd)
            prefetch(1, eng=nc.gpsimd)

            # heavy constants (needed from the first C1/h1 onward)
            Wox1_sb = pc.tile([128, H * HID], f16, tag="Wox1")
            nc.gpsimd.dma_start(out=Wox1_sb[:], in_=Wox1_d[:])
            beff1_sb = pc.tile([1, HID], f16, tag="beff1")
            nc.gpsimd.dma_start(out=beff1_sb[:], in_=beff1_d[:])
            Tm_sb = pc.tile([P, nb * T], f16, tag="Tm")
            nc.sync.dma_start(out=Tm_sb[:], in_=Tm_d[:])
            W2_sb = pc.tile([128, 4 * HID], f16, tag="W2")
            nc.gpsimd.dma_start(out=W2_sb[:], in_=W2_d[:])
            W3_sb = pc.tile([128, 4 * D], f16, tag="W3")
            nc.gpsimd.dma_start(out=W3_sb[:], in_=W3_d[:])
            b1_sb = pc.tile([128, HID // 128], f32, tag="b1")
            nc.gpsimd.dma_start(out=b1_sb[:], in_=b1_d[:])
            b2_sb = pc.tile([128, HID // 128], f32, tag="b2")
            nc.gpsimd.dma_start(out=b2_sb[:], in_=b2_d[:])
            b3_sb = pc.tile([128, 1], f32, tag="b3")
            nc.gpsimd.dma_start(out=b3_sb[:], in_=b3_d[:])
            ones24 = pc.tile([1, J], f16, tag="ones24")
            nc.vector.memset(ones24[:], 1.0)

            C1s = {}
            xTp = [None]

            def att_block(b):
                """scores -> exp -> num/den -> x -> xT -> C1_b for batch b.
                PE parts are split so exp/DVE latency hides under the h2
                stream the caller interleaves around them."""
                st = tbs[b // 2]
                off = HL * (b % 2)
                # scores: c1big opens the accumulation (t*c1 term, one
                # K=16 matmul), then 8 sin-part matmuls close per block.
                ps_s = pp.tile([128, NCH * J], f32, tag="ps_s", bufs=2,
                               name=f"ps_s_{b}")
                nc.tensor.matmul(ps_s[:], t16s[b][:], c1b_sb[:],
                                 start=True, stop=False,
                                 skip_group_check=True)
                for g in range(NG):
                    nc.tensor.matmul(ps_s[:, 2 * J * g:2 * J * (g + 1)],
                                     st[:, off + 128 * g:off + 128 * (g + 1)],
                                     As_sb[:], start=False, stop=True,
                                     skip_group_check=True)
                # w8 pads each 24-col chunk block to 32 so the DoubleRow
                # ldweights k-pair step is 16B-aligned (s3_lw restriction).
                w8 = pw.tile([128, NG * 2 * 32], f8, tag="w8")
                w8v = w8[:].rearrange("p (g k j) -> p g k j",
                                      g=NG, k=2)[:, :, :, 0:J]
                exp_i = nc.scalar.activation(
                    w8v, ps_s[:].rearrange("p (g k j) -> p g k j", g=NG, k=2),
                    AF.Exp)
                add_dep_helper(exp_i.ins, sins[-1].ins, sync=False,
                               reason="sin table set before exp set")

                def nd_block(b=b, w8v=w8v):
                    ps_nd = pp.tile([J, 2 * D], f32, tag="ps_nd", bufs=1,
                                    name=f"ps_nd_{b}")
                    V8v = V8s.pop(b)[:].rearrange("p (g k c) -> p g k c",
                                                  g=NG, k=2)
                    for g in range(NG):
                        nc.tensor.matmul(ps_nd[:], w8v[:, g], V8v[:, g],
                                         start=(g == 0), stop=(g == NG - 1),
                                         perf_mode=DRm)
                    rden = psm.tile([J, D], f32, tag="rden")
                    nc.vector.reciprocal(rden[:], ps_nd[:, D:2 * D])
                    x16 = psm.tile([J, D], f16, tag="x16")
                    nc.vector.tensor_mul(x16[:], ps_nd[:, 0:D], rden[:])
                    return x16

                def xt_block(x16, b=b):
                    ps_xt = pp.tile([D, J], f16, tag="ps_c1", bufs=1,
                                    name=f"ps_xt_{b}")
                    nc.tensor.transpose(ps_xt[:], x16[:], eye_sb[:])
                    if b % 2 == 0:
                        xTp[0] = psm.tile([D, 2 * J], f16, tag="xTp", name=f"xTp_{b}")
                    dst = xTp[0][:].rearrange("p (h c q) -> p h c q",
                                              h=H, c=2)[:, :, b % 2, :]
                    nc.vector.tensor_copy(
                        dst, ps_xt[:].rearrange("p (h q) -> p h q", h=H))

                def c1_block(b=b):
                    # C1 for the pair (b-1, b): [6, HID], rows (batch, p)
                    ps_c1 = pp.tile([2 * P, HID], f32, tag="ps_c1", bufs=1,
                                    name=f"ps_c1_{b}")
                    for h in range(H):
                        nc.tensor.matmul(ps_c1[:],
                                         xTp[0][:, 2 * P * h:2 * P * (h + 1)],
                                         Wox1_sb[:, HID * h:HID * (h + 1)],
                                         start=(h == 0), stop=False)
                    nc.tensor.matmul(ps_c1[:], ones24[:, 0:2 * P],
                                     beff1_sb[:], start=False, stop=True)
                    cp = psm.tile([2 * P, HID], f16, tag="C1p", name=f"C1p_{b}")
                    nc.vector.tensor_copy(cp[:], ps_c1[:])
                    for i, bb in enumerate((b - 1, b)):
                        cb = pc1.tile([P, HID], f16, tag=f"C1_{bb % 3}", name=f"C1_{bb}")
                        eng = nc.sync if i == 0 else nc.gpsimd
                        eng.dma_start(out=cb[:],
                                      in_=cp[P * i:P * (i + 1), :])
                        C1s[bb] = cb

                return nd_block, xt_block, (c1_block if b % 2 == 1 else None)

            # ---- fully pipelined steps ----
            h1_cur = None
            h1_prev = None
            nd_fn = xt_fn = c1_fn = None
            for s in range(-1, nb + 2):
                ba, bh1, bh2 = s + 1, s - 1, s - 2
                prefetch(s + 2)
                # attention part 1 of batch s+1 (scores + exp issued)
                if ba <= nb - 1:
                    nd_fn, xt_fn, c1_fn = att_block(ba)
                else:
                    nd_fn = xt_fn = c1_fn = None
                if 0 <= bh1 < nb:
                    h1_cur = [ph1.tile([128, T], f16, tag=f"h1_{m}", bufs=2,
                                       name=f"h1_{bh1}_{m}")
                              for m in range(4)]

                def h1_job(i, bh1=bh1, h1_cur=h1_cur):
                    m, tg = i // 2, i % 2
                    ps_h1 = pp.tile([128, 512], f32, tag="ps_big1", bufs=2,
                                    name=f"ps_h1_{bh1}_{i}")
                    nc.tensor.matmul(
                        ps_h1[:], C1s[bh1][:, 128 * m:128 * (m + 1)],
                        Tm_sb[:, T * bh1 + 512 * tg:T * bh1 + 512 * (tg + 1)],
                        start=True, stop=True)
                    dstv = h1_cur[m][:, 512 * tg:512 * (tg + 1)]
                    if i % 2 == 0:
                        nc.vector.tensor_scalar(dstv, ps_h1[:],
                                                b1_sb[:, m:m + 1], 0.0,
                                                ALU.add, ALU.max)
                    else:
                        nc.scalar.activation(dstv, ps_h1[:], AF.Relu,
                                             bias=b1_sb[:, m:m + 1])

                def h2_group(m, tg, bh2=bh2, h1_prev=h1_prev):
                    ps_h2 = pp.tile([128, 512], f32, tag="ps_big2",
                                    bufs=2, name=f"ps_h2_{bh2}_{m}_{tg}")
                    for k in range(4):
                        nc.tensor.matmul(
                            ps_h2[:],
                            W2_sb[:, HID * k + 128 * m:
                                  HID * k + 128 * (m + 1)],
                            h1_prev[k][:, 512 * tg:512 * (tg + 1)],
                            start=(k == 0), stop=(k == 3))
                    nc.scalar.activation(
                        h2s[m][:, 512 * tg:512 * (tg + 1)], ps_h2[:],
                        AF.Relu, bias=b2_sb[:, m:m + 1])

                if bh2 < 0:
                    # pipeline fill: no h2 stream yet
                    if nd_fn is not None:
                        xt_fn(nd_fn())
                        if c1_fn is not None:
                            c1_fn()
                    if 0 <= bh1 < nb:
                        for i in range(8):
                            h1_job(i)
                    h1_prev = h1_cur
                    continue

                h2s = [ph2.tile([128, T], f16, tag=f"h2_{m}", bufs=2,
                                name=f"h2_{bh2}_{m}") for m in range(4)]
                o_sb = pout.tile([128, T], f16, tag="o_sb", name=f"o3_{bh2}")

                def out_group(tg, bh2=bh2, o_sb=o_sb):
                    ps_o = pp.tile([128, 512], f32, tag="ps_big1",
                                   bufs=2, name=f"ps_o_{bh2}_{tg}")
                    for k in range(4):
                        nc.tensor.matmul(
                            ps_o[:], W3_sb[:, D * k:D * (k + 1)],
                            h2s[k][:, 512 * tg:512 * (tg + 1)],
                            start=(k == 0), stop=(k == 3))
                    nc.vector.tensor_scalar_add(
                        o_sb[:, 512 * tg:512 * (tg + 1)], ps_o[:],
                        b3_sb[:, 0:1])
                    nc.sync.dma_start(out=o_d[bh2, :, 512 * tg:512 * (tg + 1)],
                                      in_=o_sb[:, 512 * tg:512 * (tg + 1)])

                if s == nb + 1:
                    # drain step: tg-major so the out matmuls of tg=0
                    # overlap the h2 matmuls of tg=1
                    for tg in range(2):
                        for m in range(4):
                            h2_group(m, tg)
                        out_group(tg)
                    h1_prev = h1_cur
                    continue
                # first half of the h2 stream, h1 jobs interleaved in
                # stationary-sharing pairs (halves K-geometry switches)
                for m in range(2):
                    for tg in range(2):
                        h2_group(m, tg)
                    if 0 <= bh1 < nb:
                        h1_job(2 * m)
                        h1_job(2 * m + 1)
                # attention part 2 (nd needs exp, which ran during the
                # h2 groups above); the xt transpose + C1 matmuls go one
                # h2 group later so the recip/x16 DVE latency hides
                x16v = nd_fn() if nd_fn is not None else None
                for m in range(2, 4):
                    for tg in range(2):
                        h2_group(m, tg)
                    if m == 2 and x16v is not None:
                        xt_fn(x16v)
                        if c1_fn is not None:
                            c1_fn()
                    if 0 <= bh1 < nb:
                        h1_job(2 * m)
                        h1_job(2 * m + 1)
                # out^T [D, T] = W3.T @ h2 + b3 (DVE eviction, fp16)
                for tg in range(2):
                    out_group(tg)
                h1_prev = h1_cur

    nc.compile()
    return nc


def _fold_params(inp):
    """Host-side parameter folding (float64 for exactness, cast at the end)."""
    f8d = np.float64
    q = inp["query"][0].astype(f8d) @ inp["W_q"].astype(f8d) + inp["b_q"].astype(f8d)
    Wk = inp["W_k"].astype(f8d)
    ek = E // H
    A = np.zeros((E, J))
    for h in range(H):
        cols = slice(h * ek, (h + 1) * ek)
        for p in range(P):
            A[:, h * P + p] = Wk[:, cols] @ q[p, cols]
    A /= np.sqrt(ek)
    sinm = (np.arange(E) % H) == 0
    ws = inp["w_te"].astype(f8d)[sinm]
    bs = inp["b_te"].astype(f8d)[sinm]
    As = A[sinm]
    c1 = inp["w_te"].astype(f8d)[~sinm] @ A[~sinm]
    # NOTE: the per-j constant (b_te part + b_k part) cancels in num/den.
    Wo = inp["W_o"].astype(f8d)
    Wox = np.zeros((H * D, LAT))
    beff = inp["b_o"].astype(f8d).copy()
    for h in range(H):
        Wox[h * D:(h + 1) * D] = Wo[h * 2 * D:h * 2 * D + D]
        beff += Wo[h * 2 * D + D:(h + 1) * 2 * D].sum(axis=0)
    W1 = inp["W1"].astype(f8d)
    Wox1 = Wox @ W1                                   # [H*D, HID]
    beff1 = beff @ W1                                 # [HID]
    As2 = np.zeros((128, 2 * J))
    As2[0:NS, 0:J] = As
    As2[NS:128, J:2 * J] = As
    # c1big: row i = chunk i (l in [128i, 128(i+1))), block-diag c1 at
    # the ps_s column block of chunk i: 48*(i%8) + 24*(i//8).
    c1big = np.zeros((NCH, NCH * J))
    for i in range(NCH):
        base = 2 * J * (i % NG) + J * (i // NG)
        c1big[i, base:base + J] = c1
    Wox1_sb = np.zeros((128, H * HID))
    for h in range(H):
        Wox1_sb[:, HID * h:HID * (h + 1)] = Wox1[128 * h:128 * (h + 1), :]
    W2_sb = np.zeros((128, 4 * HID))
    for k in range(4):
        W2_sb[:, HID * k:HID * (k + 1)] = inp["W2"][128 * k:128 * (k + 1), :]
    W3_sb = np.zeros((128, 4 * D))
    for k in range(4):
        W3_sb[:, D * k:D * (k + 1)] = inp["W3"][128 * k:128 * (k + 1), :]
    return {
        "As": As2.astype(np.float16),
        "wsbs": np.stack([np.concatenate([ws, ws]),
                          np.concatenate([bs, bs])], axis=1).astype(np.float32),
        "c1b": c1big.astype(np.float16),
        "Wox1": Wox1_sb.astype(np.float16),
        "beff1": beff1.astype(np.float16)[None, :],
        "W2": W2_sb.astype(np.float16),
        "W3": W3_sb.astype(np.float16),
        "b1": np.ascontiguousarray(
            inp["b1"].astype(np.float32).reshape(HID // 128, 128).T),
        "b2": np.ascontiguousarray(
            inp["b2"].astype(np.float32).reshape(HID // 128, 128).T),
        "b3": inp["b3"].astype(np.float32)[:, None],
        "eye": np.eye(J, dtype=np.float16),
    }


def kernel(**inputs):
    import ml_dtypes
    from concourse.bass_utils import run_bass_kernel_spmd

    if "prog" not in _PROG_CACHE:
        _PROG_CACHE["prog"] = _build_program(
            phase=_PROG_CACHE.get("phase", 3))
    nc = _PROG_CACHE["prog"]

    inp = {k: np.asarray(v) for k, v in inputs.items()}
    params = _fold_params(inp)

    t16 = inp["timesteps"].astype(np.float16)            # [B, L]
    y16 = inp["y_time_steps"].astype(np.float16)         # [B, T]
    t2y = (y16.astype(np.float32) ** 2).astype(np.float16)
    # V = [M*X, M] packed [b, p, g, half, c] so l = 128*(g + 8*half) + p
    Vf = np.concatenate([inp["M"] * inp["X"], inp["M"]], axis=-1)  # [B,L,2D]
    Vp = Vf.reshape(B, 2, NG, 128, 2 * D).transpose(0, 3, 2, 1, 4)
    V8 = np.ascontiguousarray(Vp.reshape(B, 128, NG * 2 * 2 * D)).astype(
        ml_dtypes.float8_e4m3)

    in_maps = []
    for c in range(N_CORES):
        sl = slice(NB * c, NB * (c + 1))
        ones = np.ones((1, NB * T), np.float16)
        m = {
            "t2r": np.ascontiguousarray(t16[sl].reshape(NB, 2, L // 2)),
            "V": V8[sl],
            "Tm": np.concatenate(
                [ones, y16[sl].reshape(1, -1), t2y[sl].reshape(1, -1)],
                axis=0),
        }
        m.update(params)
        in_maps.append(m)

    res = run_bass_kernel_spmd(nc, in_maps, list(range(N_CORES)),
                               **_PROG_CACHE.get("run_kwargs", {}))
    _PROG_CACHE["last_results"] = res
    out = np.empty((B, T, D), np.float32)
    for c in range(N_CORES):
        out[NB * c:NB * (c + 1)] = (
            res.results[c]["o"].astype(np.float32).transpose(0, 2, 1))
    return out
